# revision 33
# baseline (speedup 1.0000x reference)
"""AttentionSE3 message-passing kernel for 8 Trainium2 NeuronCores.

Strategy (edge parallelism by destination-node range), v2 (bf16 + fp8 one-hots):
  - Host: sort edges by dst, shard so core m owns nodes [m*6250, (m+1)*6250)
    and exactly the edges pointing into that range. Within a core, nodes are
    processed in chunks of 128; each chunk's edges are padded to a fixed
    tile count so the device program is fully static.
  - All heavy operands ship in half/byte precision (tolerance is 2e-2):
    k, v, q, out in bf16; the one-hot gather/scatter matrices S[e,n] and
    S^T[n,e] as fp8e4 bytes (0/1 exact), used DIRECTLY as matmul lhsT
    against bf16 rhs (mixed-dtype matmul is legal when neither side is
    fp32) -- no on-device conversion work at all.
  - Device (per 128-node chunk): qe = S^T-gather of q via PE matmul into
    PSUM; ACT copies qe to SBUF as bf16 (enables the DVE 2x mode for the
    k*qe multiply); DVE computes kq and the per-head segmented reduce to
    scores; ACT computes ex = exp(score/16) (no max-subtraction: scores
    are bounded ~|2.5|, identical to reference algebraically); GPSIMD
    (with a slice on DVE for load balance) computes ex*v; accumulating PE
    matmuls S.T @ ex and S.T @ (ex*v) produce softmax denominators and
    weighted value sums in PSUM; normalize, store bf16.
"""
import math

import numpy as np

N_NODES = 50000
N_EDGES = 800000
HEADS = 8
FDIM = 256  # flattened feature dim: heads*32 == channels*val_dim
NCORES = 8
NPC = N_NODES // NCORES  # nodes per core: 6250
CHUNK = 128
NCHUNKS = math.ceil(NPC / CHUNK)  # 49
NODES_PAD = NCHUNKS * CHUNK  # 6272
SUBB = 2  # tiles per sub-batch (also the gather-pair sharing one PSUM bank)
ACC_DEFER = 10  # sub-batches to defer acc matmuls by (keeps PE stream fed)
WMUL_GPS = 13  # of the tpc tiles per chunk, how many run ex*v on GPSIMD (rest DVE)
RED2 = False  # two-stage score reduce (TT-add halves at 2x, then reduce)
NORM_ACT = False  # normalize on ACT (8 per-head scaled copies) instead of DVE
RHSP_BUFS = 10
QEP_BUFS = 8
PSQE_BUFS = 4  # qe2 PSUM tiles in flight (one bank each)
PS_ACC_BUFS = 2


def build_nc(tpc, nchunks=NCHUNKS, nodes_pad=NODES_PAD, reps=1):
    """Build the per-core Bass program. All shapes static given tpc.

    reps>1 repeats the whole computation (identical writes) -- used by
    test.py to measure pure HW time as (t_reps - t_1)/(reps-1), free of
    the ~tens-of-ms axon dispatch overhead.
    """
    import concourse.bacc as bacc
    import concourse.tile as tile
    from concourse import mybir

    f32 = mybir.dt.float32
    bf16 = mybir.dt.bfloat16
    fp8 = mybir.dt.float8e4
    epc = tpc * CHUNK
    rows = nchunks * epc

    nc = bacc.Bacc("TRN2", target_bir_lowering=False, debug=False)
    k_t = nc.dram_tensor("k", [rows, FDIM], bf16, kind="ExternalInput").ap()
    v_t = nc.dram_tensor("v", [rows, FDIM], bf16, kind="ExternalInput").ap()
    # one-hot S^T (node-major) / S (edge-major), fp8e4 bytes from the host
    st_t = nc.dram_tensor(
        "st", [nchunks * CHUNK, tpc * CHUNK], fp8, kind="ExternalInput"
    ).ap()
    s_t = nc.dram_tensor(
        "s", [nchunks * CHUNK, tpc * CHUNK], fp8, kind="ExternalInput"
    ).ap()
    q_t = nc.dram_tensor("q", [nodes_pad, FDIM], bf16, kind="ExternalInput").ap()
    o_t = nc.dram_tensor("out", [nodes_pad, FDIM], bf16, kind="ExternalOutput").ap()

    with tile.TileContext(nc) as tc:
        with (
            tc.tile_pool(name="chunks", bufs=2) as chp,
            tc.tile_pool(name="small", bufs=4) as smp,
            tc.tile_pool(name="rhsp", bufs=RHSP_BUFS) as rhsp,
            tc.tile_pool(name="qep", bufs=QEP_BUFS) as qep,
            tc.tile_pool(name="outp", bufs=2) as outp,
            tc.tile_pool(name="ps_acc", bufs=PS_ACC_BUFS, space="PSUM") as ps_acc,
            tc.tile_pool(name="ps_qe", bufs=PSQE_BUFS, space="PSUM") as ps_qe,
        ):

            for c in [c for _ in range(reps) for c in range(nchunks)]:
                # K/V rows are stored partition-major on the host:
                # row = chunk*epc + e_of*tpc + t, so each SBUF partition
                # (=e_of) reads one contiguous tpc*FDIM run per chunk.
                k_ch = chp.tile([CHUNK, tpc, FDIM], bf16, tag="k_ch")
                nc.sync.dma_start(
                    out=k_ch[:],
                    in_=k_t[c * epc : (c + 1) * epc, :].rearrange(
                        "(p t) f -> p t f", p=CHUNK
                    ),
                )
                v_ch = chp.tile([CHUNK, tpc, FDIM], bf16, tag="v_ch")
                nc.sync.dma_start(
                    out=v_ch[:],
                    in_=v_t[c * epc : (c + 1) * epc, :].rearrange(
                        "(p t) f -> p t f", p=CHUNK
                    ),
                )
                q_ch = chp.tile([CHUNK, FDIM], bf16, tag="q_ch")
                nc.sync.dma_start(out=q_ch[:], in_=q_t[c * CHUNK : (c + 1) * CHUNK, :])
                st_ch = chp.tile([CHUNK, tpc, CHUNK], fp8, tag="st_ch")
                s_ch = chp.tile([CHUNK, tpc, CHUNK], fp8, tag="s_ch")
                nc.sync.dma_start(
                    out=s_ch[:],
                    in_=s_t[c * CHUNK : (c + 1) * CHUNK, :].rearrange(
                        "p (t e) -> p t e", e=CHUNK
                    ),
                )
                nc.sync.dma_start(
                    out=st_ch[:],
                    in_=st_t[c * CHUNK : (c + 1) * CHUNK, :].rearrange(
                        "p (t e) -> p t e", e=CHUNK
                    ),
                )

                acc_s = ps_acc.tile([CHUNK, HEADS], f32, tag="acc_s")
                acc_v = ps_acc.tile([CHUNK, FDIM], f32, tag="acc_v")
                HD = FDIM // HEADS
                # sub-batches pipeline the phases: while one sub-batch runs
                # copy/mul/reduce/exp/wmul, PE continues the next one's
                # gathers. The acc matmuls for sub-batch b are EMITTED after
                # later gather phases so the in-order PE stream never stalls
                # while gather work is ready.
                deferred_acc = []

                def emit_acc(item):
                    b0, b1, rhs_sb = item
                    for t in range(b0, b1):
                        # acc_s[n,h] += S.T @ ex; acc_v[n,:] += S.T @ (ex*v)
                        nc.tensor.matmul(
                            acc_s[:],
                            lhsT=s_ch[:, t, :],
                            rhs=rhs_sb[:, t - b0, 0:HEADS],
                            start=(t == 0),
                            stop=(t == tpc - 1),
                        )
                        nc.tensor.matmul(
                            acc_v[:],
                            lhsT=s_ch[:, t, :],
                            rhs=rhs_sb[:, t - b0, HEADS:],
                            start=(t == 0),
                            stop=(t == tpc - 1),
                        )

                for bi, b0 in enumerate(range(0, tpc, SUBB)):
                    b1 = min(b0 + SUBB, tpc)
                    nb = b1 - b0
                    # per-sub-batch buffer: [:, i, 0:8] = ex, [:, i, 8:] = kq
                    # then (in-place) ex*v
                    rhs_sb = rhsp.tile([CHUNK, SUBB, HEADS + FDIM], bf16, tag="rhs_sb")
                    scores_sb = rhsp.tile([CHUNK, SUBB, HEADS], f32, tag="scores_sb")
                    # paired gathers share one PSUM bank: the first matmul's
                    # start zeroes the whole 2KB zero-region, the second
                    # accumulates into its (zeroed) half.
                    qe2 = ps_qe.tile([CHUNK, SUBB, FDIM], f32, tag="qe2")
                    for t in range(b0, b1):
                        # qe[e, f] = q_chunk[dst_local[e], f]; fp8 one-hot
                        # lhsT direct from DMA, bf16 rhs.
                        nc.tensor.matmul(
                            qe2[:, t - b0, :],
                            lhsT=st_ch[:, t, :],
                            rhs=q_ch[:],
                            start=(t == b0),
                            stop=(t == b1 - 1),
                        )
                    # PSUM f32 -> SBUF bf16 so the DVE multiply runs in 2x
                    # mode; ACT is otherwise idle so the copy is free.
                    qe_sb = qep.tile([CHUNK, SUBB, FDIM], bf16, tag="qe_sb")
                    nc.scalar.copy(out=qe_sb[:, 0:nb, :], in_=qe2[:, 0:nb, :])
                    nc.vector.tensor_mul(
                        out=rhs_sb[:, 0:nb, HEADS:],
                        in0=k_ch[:, b0:b1, :],
                        in1=qe_sb[:, 0:nb, :],
                    )
                    if len(deferred_acc) >= ACC_DEFER:
                        emit_acc(deferred_acc.pop(0))
                    if RED2:
                        # halve on DVE at 2x (bf16 TT add), then 1x reduce
                        half = rhsp.tile([CHUNK, SUBB, HEADS, HD // 2], bf16, tag="half")
                        kq4 = rhs_sb[:, 0:nb, HEADS:].rearrange(
                            "p t (h d) -> p t h d", d=HD
                        )
                        nc.vector.tensor_add(
                            out=half[:, 0:nb],
                            in0=kq4[:, :, :, 0 : HD // 2],
                            in1=kq4[:, :, :, HD // 2 : HD],
                        )
                        nc.vector.reduce_sum(
                            out=scores_sb[:, 0:nb, :],
                            in_=half[:, 0:nb],
                            axis=mybir.AxisListType.X,
                        )
                    else:
                        nc.vector.reduce_sum(
                            out=scores_sb[:, 0:nb, :],
                            in_=rhs_sb[:, 0:nb, HEADS:].rearrange(
                                "p t (h d) -> p t h d", d=HD
                            ),
                            axis=mybir.AxisListType.X,
                        )
                    nc.scalar.activation(
                        out=rhs_sb[:, 0:nb, 0:HEADS],
                        in_=scores_sb[:, 0:nb, :],
                        func=mybir.ActivationFunctionType.Exp,
                        scale=1.0 / 16.0,
                    )
                    # in-place: overwrite kq with ex*v (WAR on the reduce).
                    # Tiles [0, WMUL_GPS) of the chunk go to GPSIMD, the rest
                    # to DVE -- tile-granular engine load balance.
                    runs = []
                    for t in range(b0, b1):
                        eng = nc.gpsimd if t < WMUL_GPS else nc.vector
                        if runs and runs[-1][0] is eng:
                            runs[-1][2] = t + 1
                        else:
                            runs.append([eng, t, t + 1])
                    for eng, t0, t1 in runs:
                        nt = t1 - t0
                        i0, i1 = t0 - b0, t1 - b0
                        eng.tensor_tensor(
                            out=rhs_sb[:, i0:i1, HEADS:].rearrange(
                                "p t (h d) -> p t h d", d=HD
                            ),
                            in0=v_ch[:, t0:t1, :].rearrange(
                                "p t (h d) -> p t h d", d=HD
                            ),
                            in1=rhs_sb[:, i0:i1, 0:HEADS].unsqueeze(3).to_broadcast(
                                [CHUNK, nt, HEADS, HD]
                            ),
                            op=mybir.AluOpType.mult,
                        )
                    deferred_acc.append((b0, b1, rhs_sb))
                while deferred_acc:
                    emit_acc(deferred_acc.pop(0))

                inv = smp.tile([CHUNK, HEADS], f32, tag="inv")
                nc.vector.tensor_scalar_max(inv[:], acc_s[:], 1e-30)
                nc.vector.reciprocal(out=inv[:], in_=inv[:])
                osb = outp.tile([CHUNK, FDIM], bf16, tag="osb")
                if NORM_ACT:
                    for h in range(HEADS):
                        nc.scalar.activation(
                            out=osb[:, h * HD : (h + 1) * HD],
                            in_=acc_v[:, h * HD : (h + 1) * HD],
                            func=mybir.ActivationFunctionType.Copy,
                            scale=inv[:, h : h + 1],
                        )
                else:
                    nc.vector.tensor_tensor(
                        out=osb[:].rearrange("p (h d) -> p h d", d=HD),
                        in0=acc_v[:].rearrange("p (h d) -> p h d", d=HD),
                        in1=inv[:].unsqueeze(2).to_broadcast([CHUNK, HEADS, HD]),
                        op=mybir.AluOpType.mult,
                    )
                nc.sync.dma_start(out=o_t[c * CHUNK : (c + 1) * CHUNK, :], in_=osb[:])
    nc.compile()
    return nc


def prepare_inputs(key_edge, query_0, query_1, value, dst):
    """Host-side shard: sort edges by dst, bucket into per-core node-range
    chunks, pad each chunk to a uniform tile count. Returns (in_maps, tpc)."""
    from concourse import mybir

    bf16 = mybir.dt.np(mybir.dt.bfloat16)
    fp8 = mybir.dt.np(mybir.dt.float8e4)

    kf = np.asarray(key_edge, dtype=np.float32).reshape(N_EDGES, FDIM)
    vf = np.asarray(value, dtype=np.float32).reshape(N_EDGES, FDIM)
    q0 = np.asarray(query_0, dtype=np.float32)
    q1 = np.asarray(query_1, dtype=np.float32)
    q = np.concatenate([q0, q1], axis=-1).reshape(N_NODES, FDIM)
    dst = np.asarray(dst).astype(np.int64)

    # Balance chunk loads: assign nodes to (chunk, slot) by snake round-robin
    # over degree-sorted nodes, so every 128-node chunk gets ~mean edge count
    # and the uniform tile padding tpc = ceil(max/128) is minimal. vid is the
    # node's padded virtual id; all downstream indexing uses vid.
    G = NCORES * NCHUNKS
    deg = np.bincount(dst, minlength=N_NODES)
    nodes_sorted = np.argsort(-deg, kind="stable")
    padded = np.concatenate([nodes_sorted, np.full(G * CHUNK - N_NODES, -1)])
    grid = padded.reshape(CHUNK, G)
    grid[1::2] = grid[1::2, ::-1]  # alternate direction each round
    vid = np.empty(N_NODES, np.int64)
    rr, bb = np.nonzero(grid >= 0)
    vid[grid[rr, bb]] = bb * CHUNK + rr

    vdst = vid[dst]
    order = np.argsort(vdst, kind="stable")
    vds = vdst[order]
    g = vds // CHUNK  # global chunk id
    counts = np.bincount(g, minlength=G)
    tpc = max(1, int(math.ceil(counts.max() / CHUNK)))
    epc = tpc * CHUNK
    starts = np.concatenate([[0], np.cumsum(counts)[:-1]])
    rank = np.arange(N_EDGES) - starts[g]
    t_of = rank // CHUNK
    e_of = rank % CHUNK
    # partition-major storage: row = g*epc + e_of*tpc + t_of, so the device
    # DMA "(p t) f -> p t f" reads a contiguous tpc*FDIM run per partition.
    dest = g * epc + e_of * tpc + t_of

    rows_total = NCORES * NCHUNKS * epc
    K = np.zeros((rows_total, FDIM), bf16)
    K[dest] = kf[order].astype(bf16)
    V = np.zeros((rows_total, FDIM), bf16)
    V[dest] = vf[order].astype(bf16)
    dloc = (vds - g * CHUNK).astype(np.int64)  # 0..127 local node index
    # one-hot S (edge-major) and S^T (node-major) as fp8e4 bytes (1.0=0x38)
    st = np.zeros(G * CHUNK * tpc * CHUNK, np.uint8)
    st[((g * CHUNK + dloc) * tpc + t_of) * CHUNK + e_of] = 0x38
    st = st.reshape(G * CHUNK, tpc * CHUNK).view(fp8)
    s_oh = np.zeros(G * CHUNK * tpc * CHUNK, np.uint8)
    s_oh[((g * CHUNK + e_of) * tpc + t_of) * CHUNK + dloc] = 0x38
    s_oh = s_oh.reshape(G * CHUNK, tpc * CHUNK).view(fp8)

    qpad = np.zeros((NCORES * NODES_PAD, FDIM), bf16)
    qpad[vid] = q.astype(bf16)
    qpad = qpad.reshape(NCORES, NODES_PAD, FDIM)

    rows_core = NCHUNKS * epc
    in_maps = []
    for c in range(NCORES):
        in_maps.append(
            {
                "k": K[c * rows_core : (c + 1) * rows_core],
                "v": V[c * rows_core : (c + 1) * rows_core],
                "st": st[c * NCHUNKS * CHUNK : (c + 1) * NCHUNKS * CHUNK],
                "s": s_oh[c * NCHUNKS * CHUNK : (c + 1) * NCHUNKS * CHUNK],
                "q": qpad[c],
            }
        )
    return in_maps, tpc, vid


def combine_outputs(results, vid):
    full = np.concatenate(
        [np.asarray(r["out"], dtype=np.float32) for r in results], axis=0
    )
    return full[vid].reshape(N_NODES, FDIM // 4, 4)


def kernel(**inputs):
    from concourse.bass_utils import run_bass_kernel_spmd

    in_maps, tpc, vid = prepare_inputs(**inputs)
    nc = build_nc(tpc)
    res = run_bass_kernel_spmd(nc, in_maps, core_ids=list(range(NCORES)))
    return combine_outputs(res.results, vid)


# revision 34
# speedup vs baseline: 1.2571x; 1.2571x over previous
"""AttentionSE3 message-passing kernel for 8 Trainium2 NeuronCores.

Strategy (edge parallelism by destination-node range), v2 (bf16 + fp8 one-hots):
  - Host: sort edges by dst, shard so core m owns nodes [m*6250, (m+1)*6250)
    and exactly the edges pointing into that range. Within a core, nodes are
    processed in chunks of 128; each chunk's edges are padded to a fixed
    tile count so the device program is fully static.
  - All heavy operands ship in half/byte precision (tolerance is 2e-2):
    k, v, q, out in bf16; the one-hot gather/scatter matrices S[e,n] and
    S^T[n,e] as fp8e4 bytes (0/1 exact), used DIRECTLY as matmul lhsT
    against bf16 rhs (mixed-dtype matmul is legal when neither side is
    fp32) -- no on-device conversion work at all.
  - Device (per 128-node chunk): qe = S^T-gather of q via PE matmul into
    PSUM; ACT copies qe to SBUF as bf16 (enables the DVE 2x mode for the
    k*qe multiply); DVE computes kq and the per-head segmented reduce to
    scores; ACT computes ex = exp(score/16) (no max-subtraction: scores
    are bounded ~|2.5|, identical to reference algebraically); GPSIMD
    (with a slice on DVE for load balance) computes ex*v; accumulating PE
    matmuls S.T @ ex and S.T @ (ex*v) produce softmax denominators and
    weighted value sums in PSUM; normalize, store bf16.
"""
import math

import numpy as np

N_NODES = 50000
N_EDGES = 800000
HEADS = 8
FDIM = 256  # flattened feature dim: heads*32 == channels*val_dim
NCORES = 8
NPC = N_NODES // NCORES  # nodes per core: 6250
CHUNK = 128
NCHUNKS = math.ceil(NPC / CHUNK)  # 49
NODES_PAD = NCHUNKS * CHUNK  # 6272
SUBB = 2  # tiles per sub-batch (also the gather-pair sharing one PSUM bank)
ACC_DEFER = 8  # sub-batches to defer acc matmuls by (keeps PE stream fed)
WMUL_GPS = 13  # of the tpc tiles per chunk, how many run ex*v on GPSIMD (rest DVE)
RED2 = False  # two-stage score reduce (TT-add halves at 2x, then reduce)
NORM_ACT = False  # normalize on ACT (8 per-head scaled copies) instead of DVE
RHSP_BUFS = 8
QEP_BUFS = 6
PSQE_BUFS = 4  # qe2 PSUM tiles in flight (one bank each)
PS_ACC_BUFS = 2


def build_nc(tpc, nchunks=NCHUNKS, nodes_pad=NODES_PAD, reps=1):
    """Build the per-core Bass program. All shapes static given tpc.

    reps>1 repeats the whole computation (identical writes) -- used by
    test.py to measure pure HW time as (t_reps - t_1)/(reps-1), free of
    the ~tens-of-ms axon dispatch overhead.
    """
    import concourse.bacc as bacc
    import concourse.tile as tile
    from concourse import mybir

    f32 = mybir.dt.float32
    bf16 = mybir.dt.bfloat16
    fp8 = mybir.dt.float8e4
    epc = tpc * CHUNK
    rows = nchunks * epc

    nc = bacc.Bacc("TRN2", target_bir_lowering=False, debug=False)
    k_t = nc.dram_tensor("k", [rows, FDIM], bf16, kind="ExternalInput").ap()
    v_t = nc.dram_tensor("v", [rows, FDIM], bf16, kind="ExternalInput").ap()
    # one-hot S^T (node-major) / S (edge-major), fp8e4 bytes from the host
    st_t = nc.dram_tensor(
        "st", [nchunks * CHUNK, tpc * CHUNK], fp8, kind="ExternalInput"
    ).ap()
    s_t = nc.dram_tensor(
        "s", [nchunks * CHUNK, tpc * CHUNK], fp8, kind="ExternalInput"
    ).ap()
    q_t = nc.dram_tensor("q", [nodes_pad, FDIM], bf16, kind="ExternalInput").ap()
    o_t = nc.dram_tensor("out", [nodes_pad, FDIM], bf16, kind="ExternalOutput").ap()

    with tile.TileContext(nc) as tc:
        with (
            tc.tile_pool(name="chunks", bufs=2) as chp,
            tc.tile_pool(name="small", bufs=4) as smp,
            tc.tile_pool(name="rhsp", bufs=RHSP_BUFS) as rhsp,
            tc.tile_pool(name="qep", bufs=QEP_BUFS) as qep,
            tc.tile_pool(name="outp", bufs=2) as outp,
            tc.tile_pool(name="ps_acc", bufs=PS_ACC_BUFS, space="PSUM") as ps_acc,
            tc.tile_pool(name="ps_qe", bufs=PSQE_BUFS, space="PSUM") as ps_qe,
        ):

            for c in [c for _ in range(reps) for c in range(nchunks)]:
                # K/V rows are stored partition-major on the host:
                # row = chunk*epc + e_of*tpc + t, so each SBUF partition
                # (=e_of) reads one contiguous tpc*FDIM run per chunk.
                k_ch = chp.tile([CHUNK, tpc, FDIM], bf16, tag="k_ch")
                nc.sync.dma_start(
                    out=k_ch[:],
                    in_=k_t[c * epc : (c + 1) * epc, :].rearrange(
                        "(p t) f -> p t f", p=CHUNK
                    ),
                )
                v_ch = chp.tile([CHUNK, tpc, FDIM], bf16, tag="v_ch")
                nc.sync.dma_start(
                    out=v_ch[:],
                    in_=v_t[c * epc : (c + 1) * epc, :].rearrange(
                        "(p t) f -> p t f", p=CHUNK
                    ),
                )
                q_ch = chp.tile([CHUNK, FDIM], bf16, tag="q_ch")
                nc.sync.dma_start(out=q_ch[:], in_=q_t[c * CHUNK : (c + 1) * CHUNK, :])
                st_ch = chp.tile([CHUNK, tpc, CHUNK], fp8, tag="st_ch")
                s_ch = chp.tile([CHUNK, tpc, CHUNK], fp8, tag="s_ch")
                nc.sync.dma_start(
                    out=s_ch[:],
                    in_=s_t[c * CHUNK : (c + 1) * CHUNK, :].rearrange(
                        "p (t e) -> p t e", e=CHUNK
                    ),
                )
                nc.sync.dma_start(
                    out=st_ch[:],
                    in_=st_t[c * CHUNK : (c + 1) * CHUNK, :].rearrange(
                        "p (t e) -> p t e", e=CHUNK
                    ),
                )

                acc_s = ps_acc.tile([CHUNK, HEADS], f32, tag="acc_s")
                acc_v = ps_acc.tile([CHUNK, FDIM], f32, tag="acc_v")
                HD = FDIM // HEADS
                # sub-batches pipeline the phases: while one sub-batch runs
                # copy/mul/reduce/exp/wmul, PE continues the next one's
                # gathers. The acc matmuls for sub-batch b are EMITTED after
                # later gather phases so the in-order PE stream never stalls
                # while gather work is ready.
                deferred_acc = []

                def emit_acc(item):
                    b0, b1, rhs_sb = item
                    for t in range(b0, b1):
                        # acc_s[n,h] += S.T @ ex; acc_v[n,:] += S.T @ (ex*v)
                        nc.tensor.matmul(
                            acc_s[:],
                            lhsT=s_ch[:, t, :],
                            rhs=rhs_sb[:, t - b0, 0:HEADS],
                            start=(t == 0),
                            stop=(t == tpc - 1),
                        )
                        nc.tensor.matmul(
                            acc_v[:],
                            lhsT=s_ch[:, t, :],
                            rhs=rhs_sb[:, t - b0, HEADS:],
                            start=(t == 0),
                            stop=(t == tpc - 1),
                        )

                for bi, b0 in enumerate(range(0, tpc, SUBB)):
                    b1 = min(b0 + SUBB, tpc)
                    nb = b1 - b0
                    # per-sub-batch buffer: [:, i, 0:8] = ex, [:, i, 8:] = kq
                    # then (in-place) ex*v
                    rhs_sb = rhsp.tile([CHUNK, SUBB, HEADS + FDIM], bf16, tag="rhs_sb")
                    scores_sb = rhsp.tile([CHUNK, SUBB, HEADS], f32, tag="scores_sb")
                    # paired gathers share one PSUM bank: the first matmul's
                    # start zeroes the whole 2KB zero-region, the second
                    # accumulates into its (zeroed) half.
                    qe2 = ps_qe.tile([CHUNK, SUBB, FDIM], f32, tag="qe2")
                    for t in range(b0, b1):
                        # qe[e, f] = q_chunk[dst_local[e], f]; fp8 one-hot
                        # lhsT direct from DMA, bf16 rhs.
                        nc.tensor.matmul(
                            qe2[:, t - b0, :],
                            lhsT=st_ch[:, t, :],
                            rhs=q_ch[:],
                            start=(t == b0),
                            stop=(t == b1 - 1),
                        )
                    # PSUM f32 -> SBUF bf16 so the DVE multiply runs in 2x
                    # mode; ACT is otherwise idle so the copy is free.
                    qe_sb = qep.tile([CHUNK, SUBB, FDIM], bf16, tag="qe_sb")
                    nc.scalar.copy(out=qe_sb[:, 0:nb, :], in_=qe2[:, 0:nb, :])
                    nc.vector.tensor_mul(
                        out=rhs_sb[:, 0:nb, HEADS:],
                        in0=k_ch[:, b0:b1, :],
                        in1=qe_sb[:, 0:nb, :],
                    )
                    if len(deferred_acc) >= ACC_DEFER:
                        emit_acc(deferred_acc.pop(0))
                    if RED2:
                        # halve on DVE at 2x (bf16 TT add), then 1x reduce
                        half = rhsp.tile([CHUNK, SUBB, HEADS, HD // 2], bf16, tag="half")
                        kq4 = rhs_sb[:, 0:nb, HEADS:].rearrange(
                            "p t (h d) -> p t h d", d=HD
                        )
                        nc.vector.tensor_add(
                            out=half[:, 0:nb],
                            in0=kq4[:, :, :, 0 : HD // 2],
                            in1=kq4[:, :, :, HD // 2 : HD],
                        )
                        nc.vector.reduce_sum(
                            out=scores_sb[:, 0:nb, :],
                            in_=half[:, 0:nb],
                            axis=mybir.AxisListType.X,
                        )
                    else:
                        nc.vector.reduce_sum(
                            out=scores_sb[:, 0:nb, :],
                            in_=rhs_sb[:, 0:nb, HEADS:].rearrange(
                                "p t (h d) -> p t h d", d=HD
                            ),
                            axis=mybir.AxisListType.X,
                        )
                    nc.scalar.activation(
                        out=rhs_sb[:, 0:nb, 0:HEADS],
                        in_=scores_sb[:, 0:nb, :],
                        func=mybir.ActivationFunctionType.Exp,
                        scale=1.0 / 16.0,
                    )
                    # in-place: overwrite kq with ex*v (WAR on the reduce).
                    # Tiles [0, WMUL_GPS) of the chunk go to GPSIMD, the rest
                    # to DVE -- tile-granular engine load balance.
                    runs = []
                    for t in range(b0, b1):
                        eng = nc.gpsimd if t < WMUL_GPS else nc.vector
                        if runs and runs[-1][0] is eng:
                            runs[-1][2] = t + 1
                        else:
                            runs.append([eng, t, t + 1])
                    for eng, t0, t1 in runs:
                        nt = t1 - t0
                        i0, i1 = t0 - b0, t1 - b0
                        eng.tensor_tensor(
                            out=rhs_sb[:, i0:i1, HEADS:].rearrange(
                                "p t (h d) -> p t h d", d=HD
                            ),
                            in0=v_ch[:, t0:t1, :].rearrange(
                                "p t (h d) -> p t h d", d=HD
                            ),
                            in1=rhs_sb[:, i0:i1, 0:HEADS].unsqueeze(3).to_broadcast(
                                [CHUNK, nt, HEADS, HD]
                            ),
                            op=mybir.AluOpType.mult,
                        )
                    deferred_acc.append((b0, b1, rhs_sb))
                while deferred_acc:
                    emit_acc(deferred_acc.pop(0))

                inv = smp.tile([CHUNK, HEADS], f32, tag="inv")
                nc.vector.tensor_scalar_max(inv[:], acc_s[:], 1e-30)
                nc.vector.reciprocal(out=inv[:], in_=inv[:])
                osb = outp.tile([CHUNK, FDIM], bf16, tag="osb")
                if NORM_ACT:
                    for h in range(HEADS):
                        nc.scalar.activation(
                            out=osb[:, h * HD : (h + 1) * HD],
                            in_=acc_v[:, h * HD : (h + 1) * HD],
                            func=mybir.ActivationFunctionType.Copy,
                            scale=inv[:, h : h + 1],
                        )
                else:
                    nc.vector.tensor_tensor(
                        out=osb[:].rearrange("p (h d) -> p h d", d=HD),
                        in0=acc_v[:].rearrange("p (h d) -> p h d", d=HD),
                        in1=inv[:].unsqueeze(2).to_broadcast([CHUNK, HEADS, HD]),
                        op=mybir.AluOpType.mult,
                    )
                nc.sync.dma_start(out=o_t[c * CHUNK : (c + 1) * CHUNK, :], in_=osb[:])
    nc.compile()
    return nc


def prepare_inputs(key_edge, query_0, query_1, value, dst):
    """Host-side shard: sort edges by dst, bucket into per-core node-range
    chunks, pad each chunk to a uniform tile count. Returns (in_maps, tpc)."""
    from concourse import mybir

    bf16 = mybir.dt.np(mybir.dt.bfloat16)
    fp8 = mybir.dt.np(mybir.dt.float8e4)

    kf = np.asarray(key_edge, dtype=np.float32).reshape(N_EDGES, FDIM)
    vf = np.asarray(value, dtype=np.float32).reshape(N_EDGES, FDIM)
    q0 = np.asarray(query_0, dtype=np.float32)
    q1 = np.asarray(query_1, dtype=np.float32)
    q = np.concatenate([q0, q1], axis=-1).reshape(N_NODES, FDIM)
    dst = np.asarray(dst).astype(np.int64)

    # Balance chunk loads: assign nodes to (chunk, slot) by snake round-robin
    # over degree-sorted nodes, so every 128-node chunk gets ~mean edge count
    # and the uniform tile padding tpc = ceil(max/128) is minimal. vid is the
    # node's padded virtual id; all downstream indexing uses vid.
    G = NCORES * NCHUNKS
    deg = np.bincount(dst, minlength=N_NODES)
    nodes_sorted = np.argsort(-deg, kind="stable")
    padded = np.concatenate([nodes_sorted, np.full(G * CHUNK - N_NODES, -1)])
    grid = padded.reshape(CHUNK, G)
    grid[1::2] = grid[1::2, ::-1]  # alternate direction each round
    vid = np.empty(N_NODES, np.int64)
    rr, bb = np.nonzero(grid >= 0)
    vid[grid[rr, bb]] = bb * CHUNK + rr

    vdst = vid[dst]
    order = np.argsort(vdst, kind="stable")
    vds = vdst[order]
    g = vds // CHUNK  # global chunk id
    counts = np.bincount(g, minlength=G)
    tpc = max(1, int(math.ceil(counts.max() / CHUNK)))
    epc = tpc * CHUNK
    starts = np.concatenate([[0], np.cumsum(counts)[:-1]])
    rank = np.arange(N_EDGES) - starts[g]
    t_of = rank // CHUNK
    e_of = rank % CHUNK
    # partition-major storage: row = g*epc + e_of*tpc + t_of, so the device
    # DMA "(p t) f -> p t f" reads a contiguous tpc*FDIM run per partition.
    dest = g * epc + e_of * tpc + t_of

    rows_total = NCORES * NCHUNKS * epc
    K = np.zeros((rows_total, FDIM), bf16)
    K[dest] = kf[order].astype(bf16)
    V = np.zeros((rows_total, FDIM), bf16)
    V[dest] = vf[order].astype(bf16)
    dloc = (vds - g * CHUNK).astype(np.int64)  # 0..127 local node index
    # one-hot S (edge-major) and S^T (node-major) as fp8e4 bytes (1.0=0x38)
    st = np.zeros(G * CHUNK * tpc * CHUNK, np.uint8)
    st[((g * CHUNK + dloc) * tpc + t_of) * CHUNK + e_of] = 0x38
    st = st.reshape(G * CHUNK, tpc * CHUNK).view(fp8)
    s_oh = np.zeros(G * CHUNK * tpc * CHUNK, np.uint8)
    s_oh[((g * CHUNK + e_of) * tpc + t_of) * CHUNK + dloc] = 0x38
    s_oh = s_oh.reshape(G * CHUNK, tpc * CHUNK).view(fp8)

    qpad = np.zeros((NCORES * NODES_PAD, FDIM), bf16)
    qpad[vid] = q.astype(bf16)
    qpad = qpad.reshape(NCORES, NODES_PAD, FDIM)

    rows_core = NCHUNKS * epc
    in_maps = []
    for c in range(NCORES):
        in_maps.append(
            {
                "k": K[c * rows_core : (c + 1) * rows_core],
                "v": V[c * rows_core : (c + 1) * rows_core],
                "st": st[c * NCHUNKS * CHUNK : (c + 1) * NCHUNKS * CHUNK],
                "s": s_oh[c * NCHUNKS * CHUNK : (c + 1) * NCHUNKS * CHUNK],
                "q": qpad[c],
            }
        )
    return in_maps, tpc, vid


def combine_outputs(results, vid):
    full = np.concatenate(
        [np.asarray(r["out"], dtype=np.float32) for r in results], axis=0
    )
    return full[vid].reshape(N_NODES, FDIM // 4, 4)


def kernel(**inputs):
    from concourse.bass_utils import run_bass_kernel_spmd

    in_maps, tpc, vid = prepare_inputs(**inputs)
    nc = build_nc(tpc)
    res = run_bass_kernel_spmd(nc, in_maps, core_ids=list(range(NCORES)))
    return combine_outputs(res.results, vid)
